# revision 9
# baseline (speedup 1.0000x reference)
"""Trainium2 kernel for nn_Deblur_samll_60550448939592.

Strategy: the B1 residual-block stack (128ch, 128x128, b*t=10 images; ~290 of
~320 GFLOP) runs on the 8 NeuronCores as one SPMD Bass/Tile NEFF invoked once
per residual block. Sharding: each core computes a 16-row output band of all
10 images (rows 16c..16c+16), reading a 20-row input band (2-row halo each
side, zero-padded at image edges) so no inter-core exchange is needed inside
a block. The remaining light stages (feature-extraction convs, stage-A 32ch
64x64 resblocks, dynamic-kernel generation, dynamic depthwise convs, pixel
(un)shuffles, B2/classifier convs, sigmoids) run vectorized on host numpy.
"""

import numpy as np
from contextlib import ExitStack

import concourse.bass as bass
import concourse.tile as tile
import concourse.mybir as mybir
from concourse.bass_utils import run_bass_kernel_spmd

NF = 32
C16 = 512
K = 3
_BN_EPS = 1e-5
N_CORES = 8

# ---------------------------------------------------------------------------
# walrus workaround: this container's neuronxcc allows only ONE sem wait per
# instruction; Tile's final drain aggregates several. Hoist overflow waits
# onto preceding same-engine NOPs.
# ---------------------------------------------------------------------------


def _split_wide_waits(nc, limit=1):
    for fn in nc.m.functions:
        for bb in fn.blocks:
            insts = bb.instructions
            i = 0
            while i < len(insts):
                inst = insts[i]
                si = inst.sync_info
                if si is None:
                    i += 1
                    continue
                waits = list(si.on_wait)
                if len(waits) <= limit:
                    i += 1
                    continue
                pre, keep = waits[:-limit], waits[-limit:]
                n_nops = 0
                for j in range(0, len(pre), limit):
                    nop = mybir.InstNoOp(
                        name=f"{inst.name}_wsplit{j}",
                        engine=inst.engine,
                        sync_info=mybir.SyncInfo(
                            on_wait=pre[j : j + limit], on_update=[]
                        ),
                    )
                    insts.insert(i + n_nops, nop)
                    n_nops += 1
                si.on_wait = keep
                i += n_nops + 1


# ---------------------------------------------------------------------------
# host numpy stages
# ---------------------------------------------------------------------------


def _conv3(x, w, b):
    # x [N,C,H,W], w [O,C,3,3] -> [N,O,H,W], SAME padding
    n, c, h, wd = x.shape
    o = w.shape[0]
    xp = np.pad(x, ((0, 0), (0, 0), (1, 1), (1, 1)))
    wm = w.reshape(o, c * 9).T.astype(np.float32)  # [C*9, O]
    out = np.empty((n, o, h, wd), np.float32)
    for i in range(n):
        pat = np.lib.stride_tricks.sliding_window_view(xp[i], (3, 3), axis=(1, 2))
        # pat [C,H,W,3,3] -> [H*W, C*9]
        pm = pat.transpose(1, 2, 0, 3, 4).reshape(h * wd, c * 9)
        out[i] = (pm @ wm).T.reshape(o, h, wd)
    return out + b[None, :, None, None]


def _resblocks(x, w1, b1, w2, b2):
    for i in range(w1.shape[0]):
        y = _conv3(x, w1[i], b1[i])
        np.maximum(y, 0.0, out=y)
        x = x + _conv3(y, w2[i], b2[i])
    return x


def _pix_unshuffle(x, r):
    b, c, h, w = x.shape
    return (
        x.reshape(b, c, h // r, r, w // r, r)
        .transpose(0, 1, 3, 5, 2, 4)
        .reshape(b, c * r * r, h // r, w // r)
    )


def _pix_shuffle(x, r):
    b, c, h, w = x.shape
    return (
        x.reshape(b, c // (r * r), r, r, h, w)
        .transpose(0, 1, 4, 2, 5, 3)
        .reshape(b, c // (r * r), h * r, w * r)
    )


def _dyd_weight(S, w1, g, beta, w2, b2):
    p = S.mean(axis=(2, 3))
    hdn = np.maximum(
        (p @ w1.T) * (g / np.float32(np.sqrt(1.0 + _BN_EPS))) + beta, 0.0
    )
    return (hdn @ w2.T + b2).reshape(-1, C16, K, K).astype(np.float32)


def _dyn_dwconv(L, Wk, bias):
    # L [S,C,H,W], Wk [S,C,3,3] per-sample depthwise
    s, c, h, w = L.shape
    xp = np.pad(L, ((0, 0), (0, 0), (1, 1), (1, 1)))
    y = np.zeros_like(L)
    for dy in range(3):
        for dx in range(3):
            y += Wk[:, :, dy, dx][:, :, None, None] * xp[:, :, dy : dy + h, dx : dx + w]
    return y + bias[None, :, None, None]


def _sigmoid(x):
    return 1.0 / (1.0 + np.exp(-x))


def _resize_quarter(x):
    # jax.image.resize bilinear antialias=False, 256->64: out[i] maps to input
    # coord 4i+1.5 -> mean of pixels (4i+1, 4i+2) in each axis.
    a = 0.5 * (x[..., 1::4, :] + x[..., 2::4, :])
    return 0.5 * (a[..., :, 1::4] + a[..., :, 2::4])


# ---------------------------------------------------------------------------
# device: one 128ch residual block over [10,128,128,128], row-band sharded
# ---------------------------------------------------------------------------

_IMGS = 10
_CH = 128
_ROWS_OUT = 16  # per core
_ROWS_IN = 20  # +2 halo each side
_WPAD = 130  # 128 cols + 1 zero col each side

_cached = {}
EXEC_NS = []  # per-launch neuron-profile exec_time_ns (when tracing enabled)


def _build_block_nc():
    nc = bass.Bass(target_bir_lowering=False)
    dt = mybir.dt.float32
    xb = nc.dram_tensor("xb", [_IMGS, _CH, _ROWS_IN, 128], dt, kind="ExternalInput")
    w1T = nc.dram_tensor("w1T", [_CH, 9 * _CH], dt, kind="ExternalInput")
    w2T = nc.dram_tensor("w2T", [_CH, 9 * _CH], dt, kind="ExternalInput")
    b1 = nc.dram_tensor("b1", [_CH, 1], dt, kind="ExternalInput")
    b2 = nc.dram_tensor("b2", [_CH, 1], dt, kind="ExternalInput")
    # hz[:,0] multiplies conv1-output row 0, hz[:,1] row 17: 0.0 when that row
    # lies outside the image (conv2 must see zeros there), else 1.0.
    hz = nc.dram_tensor("hz", [_CH, 2], dt, kind="ExternalInput")
    yb = nc.dram_tensor("yb", [_IMGS, _CH, _ROWS_OUT, 128], dt, kind="ExternalOutput")

    AF = mybir.ActivationFunctionType

    with tile.TileContext(nc) as tc:
        with ExitStack() as ctx:
            wpool = ctx.enter_context(tc.tile_pool(name="w", bufs=1))
            xpool = ctx.enter_context(tc.tile_pool(name="x", bufs=2))
            hpool = ctx.enter_context(tc.tile_pool(name="h", bufs=2))
            opool = ctx.enter_context(tc.tile_pool(name="o", bufs=2))
            tpool = ctx.enter_context(tc.tile_pool(name="t", bufs=4))
            ppool = ctx.enter_context(tc.tile_pool(name="ps", bufs=4, space="PSUM"))

            w1t = wpool.tile([_CH, 9 * _CH], dt)
            nc.sync.dma_start(w1t[:], w1T[:, :])
            w2t = wpool.tile([_CH, 9 * _CH], dt)
            nc.sync.dma_start(w2t[:], w2T[:, :])
            b1t = wpool.tile([_CH, 1], dt)
            nc.sync.dma_start(b1t[:], b1[:, :])
            b2t = wpool.tile([_CH, 1], dt)
            nc.sync.dma_start(b2t[:], b2[:, :])
            hzt = wpool.tile([_CH, 2], dt)
            nc.sync.dma_start(hzt[:], hz[:, :])

            for im in range(_IMGS):
                xt = xpool.tile([_CH, _ROWS_IN, _WPAD], dt)
                nc.vector.memset(xt[:, :, 0:1], 0.0)
                nc.vector.memset(xt[:, :, 129:130], 0.0)
                nc.sync.dma_start(xt[:, :, 1:129], xb[im])

                ht = hpool.tile([_CH, _ROWS_IN - 2, _WPAD], dt)
                nc.vector.memset(ht[:, :, 0:1], 0.0)
                nc.vector.memset(ht[:, :, 129:130], 0.0)

                # conv1 + bias + relu -> ht rows 0..17 (== x rows 1..18)
                for h0, s in ((0, 4), (4, 4), (8, 4), (12, 4), (16, 2)):
                    ps = ppool.tile([_CH, 512], dt, tag="ps")
                    for dy in range(3):
                        for dx in range(3):
                            t9 = dy * 3 + dx
                            nc.tensor.matmul(
                                ps[:, : s * 128],
                                w1t[:, t9 * _CH : (t9 + 1) * _CH],
                                xt[:, h0 + dy : h0 + dy + s, dx : dx + 128],
                                start=(t9 == 0),
                                stop=(t9 == 8),
                            )
                    nc.scalar.activation(
                        ht[:, h0 : h0 + s, 1:129],
                        ps[:, : s * 128],
                        AF.Relu,
                        bias=b1t[:],
                        scale=1.0,
                    )

                nc.vector.tensor_scalar_mul(
                    ht[:, 0:1, 1:129], ht[:, 0:1, 1:129], hzt[:, 0:1]
                )
                nc.vector.tensor_scalar_mul(
                    ht[:, 17:18, 1:129], ht[:, 17:18, 1:129], hzt[:, 1:2]
                )

                ot = opool.tile([_CH, _ROWS_OUT, 128], dt)
                # conv2 + bias + residual -> out rows o (== x rows o+2)
                for o0 in (0, 4, 8, 12):
                    ps = ppool.tile([_CH, 512], dt, tag="ps")
                    for dy in range(3):
                        for dx in range(3):
                            t9 = dy * 3 + dx
                            nc.tensor.matmul(
                                ps[:, :512],
                                w2t[:, t9 * _CH : (t9 + 1) * _CH],
                                ht[:, o0 + dy : o0 + dy + 4, dx : dx + 128],
                                start=(t9 == 0),
                                stop=(t9 == 8),
                            )
                    tt = tpool.tile([_CH, 512], dt, tag="tt")
                    nc.scalar.activation(
                        tt[:], ps[:, :512], AF.Identity, bias=b2t[:], scale=1.0
                    )
                    nc.vector.tensor_add(
                        ot[:, o0 : o0 + 4, :],
                        tt[:].rearrange("p (r c) -> p r c", r=4),
                        xt[:, o0 + 2 : o0 + 6, 1:129],
                    )
                nc.sync.dma_start(yb[im], ot[:])

    _split_wide_waits(nc)
    return nc


def _run_block_device(x_full, w1, b1, w2, b2):
    """x_full [10,128,128,128] -> x + conv2(relu(conv1(x))) on 8 cores."""
    if "nc" not in _cached:
        _cached["nc"] = _build_block_nc()
    nc = _cached["nc"]

    w1T = np.ascontiguousarray(
        w1.transpose(1, 2, 3, 0).reshape(_CH, 9 * _CH).astype(np.float32)
    )
    w2T = np.ascontiguousarray(
        w2.transpose(1, 2, 3, 0).reshape(_CH, 9 * _CH).astype(np.float32)
    )
    b1c = np.ascontiguousarray(b1.reshape(_CH, 1).astype(np.float32))
    b2c = np.ascontiguousarray(b2.reshape(_CH, 1).astype(np.float32))

    xp = np.pad(x_full, ((0, 0), (0, 0), (2, 2), (0, 0)))  # rows padded by 2
    in_maps = []
    for c in range(N_CORES):
        r0 = 16 * c  # padded coords: rows r0..r0+20 cover out rows 16c..16c+16
        band = np.ascontiguousarray(xp[:, :, r0 : r0 + _ROWS_IN, :])
        hzc = np.ones((_CH, 2), np.float32)
        if c == 0:
            hzc[:, 0] = 0.0  # conv1 row 0 = image row -1
        if c == N_CORES - 1:
            hzc[:, 1] = 0.0  # conv1 row 17 = image row 128
        in_maps.append(
            {"xb": band, "w1T": w1T, "w2T": w2T, "b1": b1c, "b2": b2c, "hz": hzc}
        )

    import time as _time

    _t0 = _time.time()
    res = run_bass_kernel_spmd(nc, in_maps, core_ids=list(range(N_CORES)))
    _launch_s = _time.time() - _t0
    EXEC_NS.append(
        res.exec_time_ns if res.exec_time_ns else int(_launch_s * 1e9)
    )
    out = np.concatenate([res.results[c]["yb"] for c in range(N_CORES)], axis=2)
    return out


USE_DEVICE = True


def kernel(lrs, fe_w, fe_b, fe1_w, fe1_b,
           A1_w1, A1_b1, A1_w2, A1_b2,
           A2_w1, A2_b1, A2_w2, A2_b2,
           B1_w1, B1_b1, B1_w2, B1_b2,
           B2_w1, B2_b1, B2_w2, B2_b2,
           cl_w, cl_b, cl1_w, cl1_b,
           dyd_w1, dyd_g, dyd_beta, dyd_w2, dyd_b2, dyn_bias):
    lrs = np.asarray(lrs, np.float32)
    b, t, c, h, w = lrs.shape

    lrs_mid4 = _resize_quarter(lrs[:, t // 2])
    x0 = _conv3(lrs_mid4, fe_w, fe_b)
    feats = _conv3(lrs.reshape(b * t, c, h, w), fe1_w, fe1_b)
    x00 = _resblocks(x0, A1_w1, A1_b1, A1_w2, A1_b2)

    Wk = _dyd_weight(_pix_unshuffle(x00, 4), dyd_w1, dyd_g, dyd_beta, dyd_w2, dyd_b2)
    mid = _dyn_dwconv(_pix_unshuffle(feats, 4), np.repeat(Wk, t, axis=0), dyn_bias)

    fB = _pix_shuffle(mid, 2)
    if USE_DEVICE:
        for i in range(B1_w1.shape[0]):
            fB = _run_block_device(fB, B1_w1[i], B1_b1[i], B1_w2[i], B1_b2[i])
    else:
        fB = _resblocks(fB, B1_w1, B1_b1, B1_w2, B1_b2)

    delivery = _resblocks(x00, A2_w1, A2_b1, A2_w2, A2_b2)
    restore4 = _sigmoid(_conv3(delivery, cl_w, cl_b))

    Wk2 = _dyd_weight(
        _pix_unshuffle(delivery, 4), dyd_w1, dyd_g, dyd_beta, dyd_w2, dyd_b2
    )
    mid2 = _dyn_dwconv(_pix_unshuffle(fB, 2), np.repeat(Wk2, t, axis=0), dyn_bias)
    f2 = _resblocks(_pix_shuffle(mid2, 4), B2_w1, B2_b1, B2_w2, B2_b2)
    out = _sigmoid(_conv3(f2, cl1_w, cl1_b)).reshape(b, t, 3, h, w)
    return out.astype(np.float32), restore4.astype(np.float32)


# revision 10
# speedup vs baseline: 16.3495x; 16.3495x over previous
"""Trainium2 kernel for nn_Deblur_samll_60550448939592.

Strategy: the B1 residual-block stack (128ch, 128x128, b*t=10 images; ~290 of
~320 GFLOP) runs on the 8 NeuronCores as one SPMD Bass/Tile NEFF invoked once
per residual block. Sharding: each core computes a 16-row output band of all
10 images (rows 16c..16c+16), reading a 20-row input band (2-row halo each
side, zero-padded at image edges) so no inter-core exchange is needed inside
a block. The remaining light stages (feature-extraction convs, stage-A 32ch
64x64 resblocks, dynamic-kernel generation, dynamic depthwise convs, pixel
(un)shuffles, B2/classifier convs, sigmoids) run vectorized on host numpy.
"""

import numpy as np
from contextlib import ExitStack

import jax

# Persist compiled executables (incl. the NEFF custom-call) across processes
# so only the very first run on a machine pays the neuronxcc compile.
jax.config.update("jax_compilation_cache_dir", "/tmp/jax_comp_cache")
jax.config.update("jax_persistent_cache_min_compile_time_secs", 1.0)
jax.config.update("jax_persistent_cache_min_entry_size_bytes", 0)

import concourse.bass as bass
import concourse.tile as tile
import concourse.mybir as mybir
from concourse.bass_utils import run_bass_kernel_spmd

NF = 32
C16 = 512
K = 3
_BN_EPS = 1e-5
N_CORES = 8

# ---------------------------------------------------------------------------
# walrus workaround: this container's neuronxcc allows only ONE sem wait per
# instruction; Tile's final drain aggregates several. Hoist overflow waits
# onto preceding same-engine NOPs.
# ---------------------------------------------------------------------------


def _split_wide_waits(nc, limit=1):
    for fn in nc.m.functions:
        for bb in fn.blocks:
            insts = bb.instructions
            i = 0
            while i < len(insts):
                inst = insts[i]
                si = inst.sync_info
                if si is None:
                    i += 1
                    continue
                waits = list(si.on_wait)
                if len(waits) <= limit:
                    i += 1
                    continue
                pre, keep = waits[:-limit], waits[-limit:]
                n_nops = 0
                for j in range(0, len(pre), limit):
                    nop = mybir.InstNoOp(
                        name=f"{inst.name}_wsplit{j}",
                        engine=inst.engine,
                        sync_info=mybir.SyncInfo(
                            on_wait=pre[j : j + limit], on_update=[]
                        ),
                    )
                    insts.insert(i + n_nops, nop)
                    n_nops += 1
                si.on_wait = keep
                i += n_nops + 1


# ---------------------------------------------------------------------------
# host numpy stages
# ---------------------------------------------------------------------------


def _conv3(x, w, b):
    # x [N,C,H,W], w [O,C,3,3] -> [N,O,H,W], SAME padding
    n, c, h, wd = x.shape
    o = w.shape[0]
    xp = np.pad(x, ((0, 0), (0, 0), (1, 1), (1, 1)))
    wm = w.reshape(o, c * 9).T.astype(np.float32)  # [C*9, O]
    out = np.empty((n, o, h, wd), np.float32)
    for i in range(n):
        pat = np.lib.stride_tricks.sliding_window_view(xp[i], (3, 3), axis=(1, 2))
        # pat [C,H,W,3,3] -> [H*W, C*9]
        pm = pat.transpose(1, 2, 0, 3, 4).reshape(h * wd, c * 9)
        out[i] = (pm @ wm).T.reshape(o, h, wd)
    return out + b[None, :, None, None]


def _resblocks(x, w1, b1, w2, b2):
    for i in range(w1.shape[0]):
        y = _conv3(x, w1[i], b1[i])
        np.maximum(y, 0.0, out=y)
        x = x + _conv3(y, w2[i], b2[i])
    return x


def _pix_unshuffle(x, r):
    b, c, h, w = x.shape
    return (
        x.reshape(b, c, h // r, r, w // r, r)
        .transpose(0, 1, 3, 5, 2, 4)
        .reshape(b, c * r * r, h // r, w // r)
    )


def _pix_shuffle(x, r):
    b, c, h, w = x.shape
    return (
        x.reshape(b, c // (r * r), r, r, h, w)
        .transpose(0, 1, 4, 2, 5, 3)
        .reshape(b, c // (r * r), h * r, w * r)
    )


def _dyd_weight(S, w1, g, beta, w2, b2):
    p = S.mean(axis=(2, 3))
    hdn = np.maximum(
        (p @ w1.T) * (g / np.float32(np.sqrt(1.0 + _BN_EPS))) + beta, 0.0
    )
    return (hdn @ w2.T + b2).reshape(-1, C16, K, K).astype(np.float32)


def _dyn_dwconv(L, Wk, bias):
    # L [S,C,H,W], Wk [S,C,3,3] per-sample depthwise
    s, c, h, w = L.shape
    xp = np.pad(L, ((0, 0), (0, 0), (1, 1), (1, 1)))
    y = np.zeros_like(L)
    for dy in range(3):
        for dx in range(3):
            y += Wk[:, :, dy, dx][:, :, None, None] * xp[:, :, dy : dy + h, dx : dx + w]
    return y + bias[None, :, None, None]


def _sigmoid(x):
    return 1.0 / (1.0 + np.exp(-x))


def _resize_quarter(x):
    # jax.image.resize bilinear antialias=False, 256->64: out[i] maps to input
    # coord 4i+1.5 -> mean of pixels (4i+1, 4i+2) in each axis.
    a = 0.5 * (x[..., 1::4, :] + x[..., 2::4, :])
    return 0.5 * (a[..., :, 1::4] + a[..., :, 2::4])


# ---------------------------------------------------------------------------
# device: one 128ch residual block over [10,128,128,128], row-band sharded
# ---------------------------------------------------------------------------

_IMGS = 10
_CH = 128
_ROWS_OUT = 16  # per core
_ROWS_IN = 20  # +2 halo each side
_WPAD = 130  # 128 cols + 1 zero col each side

_cached = {}
EXEC_NS = []  # per-launch neuron-profile exec_time_ns (when tracing enabled)


def _build_block_nc():
    nc = bass.Bass(target_bir_lowering=False)
    dt = mybir.dt.float32
    xb = nc.dram_tensor("xb", [_IMGS, _CH, _ROWS_IN, 128], dt, kind="ExternalInput")
    w1T = nc.dram_tensor("w1T", [_CH, 9 * _CH], dt, kind="ExternalInput")
    w2T = nc.dram_tensor("w2T", [_CH, 9 * _CH], dt, kind="ExternalInput")
    b1 = nc.dram_tensor("b1", [_CH, 1], dt, kind="ExternalInput")
    b2 = nc.dram_tensor("b2", [_CH, 1], dt, kind="ExternalInput")
    # hz[:,0] multiplies conv1-output row 0, hz[:,1] row 17: 0.0 when that row
    # lies outside the image (conv2 must see zeros there), else 1.0.
    hz = nc.dram_tensor("hz", [_CH, 2], dt, kind="ExternalInput")
    yb = nc.dram_tensor("yb", [_IMGS, _CH, _ROWS_OUT, 128], dt, kind="ExternalOutput")

    AF = mybir.ActivationFunctionType

    with tile.TileContext(nc) as tc:
        with ExitStack() as ctx:
            wpool = ctx.enter_context(tc.tile_pool(name="w", bufs=1))
            xpool = ctx.enter_context(tc.tile_pool(name="x", bufs=2))
            hpool = ctx.enter_context(tc.tile_pool(name="h", bufs=2))
            opool = ctx.enter_context(tc.tile_pool(name="o", bufs=2))
            tpool = ctx.enter_context(tc.tile_pool(name="t", bufs=4))
            ppool = ctx.enter_context(tc.tile_pool(name="ps", bufs=4, space="PSUM"))

            w1t = wpool.tile([_CH, 9 * _CH], dt)
            nc.sync.dma_start(w1t[:], w1T[:, :])
            w2t = wpool.tile([_CH, 9 * _CH], dt)
            nc.sync.dma_start(w2t[:], w2T[:, :])
            b1t = wpool.tile([_CH, 1], dt)
            nc.sync.dma_start(b1t[:], b1[:, :])
            b2t = wpool.tile([_CH, 1], dt)
            nc.sync.dma_start(b2t[:], b2[:, :])
            hzt = wpool.tile([_CH, 2], dt)
            nc.sync.dma_start(hzt[:], hz[:, :])

            for im in range(_IMGS):
                xt = xpool.tile([_CH, _ROWS_IN, _WPAD], dt)
                nc.vector.memset(xt[:, :, 0:1], 0.0)
                nc.vector.memset(xt[:, :, 129:130], 0.0)
                nc.sync.dma_start(xt[:, :, 1:129], xb[im])

                ht = hpool.tile([_CH, _ROWS_IN - 2, _WPAD], dt)
                nc.vector.memset(ht[:, :, 0:1], 0.0)
                nc.vector.memset(ht[:, :, 129:130], 0.0)

                # conv1 + bias + relu -> ht rows 0..17 (== x rows 1..18)
                for h0, s in ((0, 4), (4, 4), (8, 4), (12, 4), (16, 2)):
                    ps = ppool.tile([_CH, 512], dt, tag="ps")
                    for dy in range(3):
                        for dx in range(3):
                            t9 = dy * 3 + dx
                            nc.tensor.matmul(
                                ps[:, : s * 128],
                                w1t[:, t9 * _CH : (t9 + 1) * _CH],
                                xt[:, h0 + dy : h0 + dy + s, dx : dx + 128],
                                start=(t9 == 0),
                                stop=(t9 == 8),
                            )
                    nc.scalar.activation(
                        ht[:, h0 : h0 + s, 1:129],
                        ps[:, : s * 128],
                        AF.Relu,
                        bias=b1t[:],
                        scale=1.0,
                    )

                nc.vector.tensor_scalar_mul(
                    ht[:, 0:1, 1:129], ht[:, 0:1, 1:129], hzt[:, 0:1]
                )
                nc.vector.tensor_scalar_mul(
                    ht[:, 17:18, 1:129], ht[:, 17:18, 1:129], hzt[:, 1:2]
                )

                ot = opool.tile([_CH, _ROWS_OUT, 128], dt)
                # conv2 + bias + residual -> out rows o (== x rows o+2)
                for o0 in (0, 4, 8, 12):
                    ps = ppool.tile([_CH, 512], dt, tag="ps")
                    for dy in range(3):
                        for dx in range(3):
                            t9 = dy * 3 + dx
                            nc.tensor.matmul(
                                ps[:, :512],
                                w2t[:, t9 * _CH : (t9 + 1) * _CH],
                                ht[:, o0 + dy : o0 + dy + 4, dx : dx + 128],
                                start=(t9 == 0),
                                stop=(t9 == 8),
                            )
                    tt = tpool.tile([_CH, 512], dt, tag="tt")
                    nc.scalar.activation(
                        tt[:], ps[:, :512], AF.Identity, bias=b2t[:], scale=1.0
                    )
                    nc.vector.tensor_add(
                        ot[:, o0 : o0 + 4, :],
                        tt[:].rearrange("p (r c) -> p r c", r=4),
                        xt[:, o0 + 2 : o0 + 6, 1:129],
                    )
                nc.sync.dma_start(yb[im], ot[:])

    _split_wide_waits(nc)
    return nc


def _run_block_device(x_full, w1, b1, w2, b2):
    """x_full [10,128,128,128] -> x + conv2(relu(conv1(x))) on 8 cores."""
    if "nc" not in _cached:
        _cached["nc"] = _build_block_nc()
    nc = _cached["nc"]

    w1T = np.ascontiguousarray(
        w1.transpose(1, 2, 3, 0).reshape(_CH, 9 * _CH).astype(np.float32)
    )
    w2T = np.ascontiguousarray(
        w2.transpose(1, 2, 3, 0).reshape(_CH, 9 * _CH).astype(np.float32)
    )
    b1c = np.ascontiguousarray(b1.reshape(_CH, 1).astype(np.float32))
    b2c = np.ascontiguousarray(b2.reshape(_CH, 1).astype(np.float32))

    xp = np.pad(x_full, ((0, 0), (0, 0), (2, 2), (0, 0)))  # rows padded by 2
    in_maps = []
    for c in range(N_CORES):
        r0 = 16 * c  # padded coords: rows r0..r0+20 cover out rows 16c..16c+16
        band = np.ascontiguousarray(xp[:, :, r0 : r0 + _ROWS_IN, :])
        hzc = np.ones((_CH, 2), np.float32)
        if c == 0:
            hzc[:, 0] = 0.0  # conv1 row 0 = image row -1
        if c == N_CORES - 1:
            hzc[:, 1] = 0.0  # conv1 row 17 = image row 128
        in_maps.append(
            {"xb": band, "w1T": w1T, "w2T": w2T, "b1": b1c, "b2": b2c, "hz": hzc}
        )

    import time as _time

    _t0 = _time.time()
    res = run_bass_kernel_spmd(nc, in_maps, core_ids=list(range(N_CORES)))
    _launch_s = _time.time() - _t0
    EXEC_NS.append(
        res.exec_time_ns if res.exec_time_ns else int(_launch_s * 1e9)
    )
    out = np.concatenate([res.results[c]["yb"] for c in range(N_CORES)], axis=2)
    return out


USE_DEVICE = True


def kernel(lrs, fe_w, fe_b, fe1_w, fe1_b,
           A1_w1, A1_b1, A1_w2, A1_b2,
           A2_w1, A2_b1, A2_w2, A2_b2,
           B1_w1, B1_b1, B1_w2, B1_b2,
           B2_w1, B2_b1, B2_w2, B2_b2,
           cl_w, cl_b, cl1_w, cl1_b,
           dyd_w1, dyd_g, dyd_beta, dyd_w2, dyd_b2, dyn_bias):
    lrs = np.asarray(lrs, np.float32)
    b, t, c, h, w = lrs.shape

    lrs_mid4 = _resize_quarter(lrs[:, t // 2])
    x0 = _conv3(lrs_mid4, fe_w, fe_b)
    feats = _conv3(lrs.reshape(b * t, c, h, w), fe1_w, fe1_b)
    x00 = _resblocks(x0, A1_w1, A1_b1, A1_w2, A1_b2)

    Wk = _dyd_weight(_pix_unshuffle(x00, 4), dyd_w1, dyd_g, dyd_beta, dyd_w2, dyd_b2)
    mid = _dyn_dwconv(_pix_unshuffle(feats, 4), np.repeat(Wk, t, axis=0), dyn_bias)

    fB = _pix_shuffle(mid, 2)
    if USE_DEVICE:
        for i in range(B1_w1.shape[0]):
            fB = _run_block_device(fB, B1_w1[i], B1_b1[i], B1_w2[i], B1_b2[i])
    else:
        fB = _resblocks(fB, B1_w1, B1_b1, B1_w2, B1_b2)

    delivery = _resblocks(x00, A2_w1, A2_b1, A2_w2, A2_b2)
    restore4 = _sigmoid(_conv3(delivery, cl_w, cl_b))

    Wk2 = _dyd_weight(
        _pix_unshuffle(delivery, 4), dyd_w1, dyd_g, dyd_beta, dyd_w2, dyd_b2
    )
    mid2 = _dyn_dwconv(_pix_unshuffle(fB, 2), np.repeat(Wk2, t, axis=0), dyn_bias)
    f2 = _resblocks(_pix_shuffle(mid2, 4), B2_w1, B2_b1, B2_w2, B2_b2)
    out = _sigmoid(_conv3(f2, cl1_w, cl1_b)).reshape(b, t, 3, h, w)
    return out.astype(np.float32), restore4.astype(np.float32)
